# revision 43
# baseline (speedup 1.0000x reference)
"""Multi-head attention (B=2, T=2048, d_model=1024, H=16, hd=64) on 8 Trainium2
NeuronCores.

Sharding: the 32 (batch, head) attention units are split as 4 consecutive heads
of one batch per core (core c -> batch c//4, heads 4*(c%4) .. 4*(c%4)+3). Each
core computes its own QKV projection slice, causal attention for its heads, and
a partial out-projection (its 256 rows of W_out). The host sums the 4 partials
per batch and adds b_out.

Device-side layout (everything flows transposed so no on-chip transposes are
needed until the attention output):
  qT/kT [hd, T]  <- lhsT=W slice, rhs=xT
  v     [T, hd]  (+ ones column for the row-sum trick)
  sT    [k, q]   <- lhsT=kT chunk, rhs=qT          (psum, fp32)
  E     [k, q]   <- exp(sT * 1/sqrt(hd)) on ScalarE (bf16)
  pv    [q, hd+1]<- lhsT=E chunk, rhs=[v|1]        (col hd = row sum)
  a     [q, hd]  = pv[:, :hd] * (1/pv[:, hd])      (per-partition scalar)
  aT    [hd, T]  via DRAM round-trip DMA transpose
  out  += aT.T @ W_out slice                        (partial, fp32)

Schedule: the attention phase is ACT-bound (exp is ~45% of all engine work),
so emission is software-pipelined per (q-group, head-pair) unit: scores+exp of
unit i are emitted together with the PV matmuls of unit i-1 and the out-
projection of earlier-finalized q-groups, with the remaining QKV-projection and
v tiles spread between score tiles as PE filler. Group order [1,3,2,0] puts a
cheap group last to shrink the serial tail. Diagonal q-blocks are trimmed from
both the score matmuls and the exp.
"""

import math
import os
from contextlib import ExitStack
from dataclasses import dataclass

import numpy as np
import ml_dtypes

import concourse.bass as bass
import concourse.tile as tile
from concourse import bacc, mybir
from concourse import bass_utils

AF = mybir.ActivationFunctionType
ALU = mybir.AluOpType
DT = mybir.dt

N_CORES = 8
NEG = -1e9


@dataclass(frozen=True)
class Cfg:
    T: int = 2048        # sequence length
    DM: int = 1024       # d_model
    HD: int = 64         # head dim
    NH: int = 4          # heads per core
    mode: str = "causal"  # "causal" | "full" | "bias"
    mm: str = "bf16"     # matmul operand dtype: "bf16" | "f32r" | "f32"
    # K=64 score matmuls (PE quadrant) instead of zero-padded K=128
    k64: bool = os.environ.get("MHA_K64", "1") == "1"

    @property
    def NHD(self):
        return self.NH * self.HD          # qkv slice width per core

    @property
    def KC(self):
        return self.DM // 128             # contraction chunks for projections

    @property
    def MC(self):
        return self.NHD // 128            # qT/kT partition chunks

    @property
    def TC(self):
        return self.T // 128              # t chunks

    @property
    def QW(self):
        return min(512, self.T)           # q group width

    @property
    def QG(self):
        return self.T // self.QW

    @property
    def QT(self):
        return self.QW // 128             # q tiles per group

    @property
    def mmdt(self):
        return {"bf16": DT.bfloat16, "f32r": DT.float32r, "f32": DT.float32}[self.mm]

    @property
    def npmm(self):
        return ml_dtypes.bfloat16 if self.mm == "bf16" else np.float32


def build_program(cfg: Cfg):
    """Build + compile the SPMD single-core program. Returns (nc, input_names)."""
    c = cfg
    assert c.DM % 128 == 0 and c.NHD % 128 == 0 and c.T % 512 == 0
    nc = bacc.Bacc("TRN2", target_bir_lowering=False, debug=False,
                   num_devices=N_CORES)
    f32 = DT.float32
    mmdt = c.mmdt

    xT = nc.dram_tensor("xT", [c.DM, c.T], mmdt, kind="ExternalInput").ap()
    wq = nc.dram_tensor("wq", [c.DM, c.NHD], mmdt, kind="ExternalInput").ap()
    wk = nc.dram_tensor("wk", [c.DM, c.NHD], mmdt, kind="ExternalInput").ap()
    wv = nc.dram_tensor("wv", [c.DM, c.NHD], mmdt, kind="ExternalInput").ap()
    # bq | bk | bvb packed: tiny strided DMAs cost ~2.6us of descriptor time
    biases = nc.dram_tensor("biases", [128, 2 * c.MC + c.NHD], f32,
                            kind="ExternalInput").ap()
    wo = nc.dram_tensor("wo", [c.NHD, c.DM], mmdt, kind="ExternalInput").ap()
    identd = nc.dram_tensor("ident", [128, 128], DT.bfloat16,
                            kind="ExternalInput").ap()
    maskb = None
    if c.mode == "bias":
        # additive bias, transposed: maskb[k, q]
        maskb = nc.dram_tensor("maskb", [c.T, c.T], f32, kind="ExternalInput").ap()
    # bf16 partials: host upconverts and sums in f32
    out = nc.dram_tensor("out", [c.T, c.DM], DT.bfloat16,
                         kind="ExternalOutput").ap()

    with tile.TileContext(nc) as tc, ExitStack() as ctx:
        _body(ctx, tc, c, xT, wq, wk, wv, biases, wo, identd, maskb, out)
    nc.compile()
    names = ["xT", "wq", "wk", "wv", "biases", "wo", "ident"]
    if c.mode == "bias":
        names.append("maskb")
    return nc, names


def _body(ctx, tc, c: Cfg, xT, wq, wk, wv, biases, wo, identd, maskb, out):
    nc = tc.nc
    f32 = DT.float32
    bf16 = DT.bfloat16
    mmdt = c.mmdt
    causal = c.mode == "causal"
    scale = 1.0 / math.sqrt(c.HD)
    NHP = c.NH // 2                      # head pairs per core
    HD1 = c.HD + 1
    W2 = min(1024, c.T)                  # qk projection block width
    NB = c.T // W2                       # qk projection blocks
    VG = min(c.TC, max(1, 1024 // c.NHD))  # t-chunks per v psum tile
    NVT = c.TC // VG

    const = ctx.enter_context(tc.tile_pool(name="const", bufs=1))
    big = ctx.enter_context(tc.tile_pool(name="big", bufs=1))
    # lag-1 pipeline: E tiles of two consecutive units are alive at once
    # (worst case 2 * kmax_max/2 * 2 = 32 for causal T=2048)
    epool = ctx.enter_context(
        tc.tile_pool(name="E", bufs=(8 * c.QT if causal else c.TC + 4)))
    rpool = ctx.enter_context(tc.tile_pool(name="r", bufs=8))
    # PSUM: 3 x [128,1024] (6 banks) + 2 x [128,260] (2 banks) = 8 banks
    ps_mm = ctx.enter_context(tc.tile_pool(name="psmm", bufs=3, space="PSUM"))
    ps_pv = ctx.enter_context(tc.tile_pool(name="pspv", bufs=2, space="PSUM"))
    ostage = ctx.enter_context(tc.tile_pool(name="ostage", bufs=3))
    bias_pool = None
    if c.mode == "bias":
        bias_pool = ctx.enter_context(tc.tile_pool(name="maskb", bufs=4))

    # ---- input DMAs: qk weights first, then xT in pieces (the first score
    # block only needs t<512), with everything ordered by first use so the
    # sync queue's ~0.8us/descriptor serialization doesn't delay the PE ----
    w_sbs = {}
    for nm, w in (("wq", wq), ("wk", wk), ("wv", wv)):
        w_sbs[nm] = big.tile([128, c.KC, c.NHD], mmdt, tag=nm, name=nm)
    wo_sb = big.tile([128, c.MC, c.DM], mmdt, tag="wo")
    xT_sb = big.tile([128, c.KC, c.T], mmdt, tag="xT")
    xTd = xT.rearrange("(c p) t -> p c t", p=128)
    bias_sb = const.tile([128, 2 * c.MC + c.NHD], f32, tag="bias")
    bq_sb = bias_sb[:, 0:c.MC]
    bk_sb = bias_sb[:, c.MC:2 * c.MC]
    bvb_sb = bias_sb[:, 2 * c.MC:2 * c.MC + c.NHD]

    wqd = wq.rearrange("(c p) n -> p c n", p=128)
    wkd = wk.rearrange("(c p) n -> p c n", p=128)
    # sync queue: exactly what the prelude + early score units need, in
    # first-use order (the first tile needs wq/wk m=0 halves + xT[:, :, 0:W2])
    nc.sync.dma_start(out=w_sbs["wq"][:, :, 0:128], in_=wqd[:, :, 0:128])
    nc.sync.dma_start(out=w_sbs["wk"][:, :, 0:128], in_=wkd[:, :, 0:128])
    xcuts = ([0, c.T // 4, c.T // 2, 3 * c.T // 4, c.T]
             if c.T >= 2048 else [0, c.T])
    for x0, x1 in zip(xcuts[:2], xcuts[1:3]):
        nc.sync.dma_start(out=xT_sb[:, :, x0:x1], in_=xTd[:, :, x0:x1])
    nc.sync.dma_start(out=w_sbs["wq"][:, :, 128:c.NHD],
                      in_=wqd[:, :, 128:c.NHD])
    nc.sync.dma_start(out=w_sbs["wk"][:, :, 128:c.NHD],
                      in_=wkd[:, :, 128:c.NHD])
    nc.sync.dma_start(out=w_sbs["wv"][:],
                      in_=wv.rearrange("(c p) n -> p c n", p=128))
    nc.sync.dma_start(out=bias_sb[:], in_=biases)
    for x0, x1 in zip(xcuts[2:], xcuts[3:]):
        nc.sync.dma_start(out=xT_sb[:, :, x0:x1], in_=xTd[:, :, x0:x1])
    nc.sync.dma_start(out=wo_sb[:],
                      in_=wo.rearrange("(c p) n -> p c n", p=128))
    wq_sb, wk_sb, wv_sb = w_sbs["wq"], w_sbs["wk"], w_sbs["wv"]

    # causal mask block for diagonal tiles: tri[k, j] = 0 if j >= k else NEG
    tri = const.tile([128, 128], f32, tag="tri")
    nc.gpsimd.memset(tri[:], 0.0)
    nc.gpsimd.affine_select(
        out=tri[:], in_=tri[:],
        compare_op=ALU.is_ge, fill=NEG,
        base=0, channel_multiplier=-1, pattern=[[1, 128]],
    )
    # identity (bf16) for PE transposes of the attention output (host input)
    ident = const.tile([128, 128], bf16, tag="ident")
    nc.sync.dma_start(out=ident[:], in_=identd)

    # k64: qT packed like kT ([128, MC, T], head pair per 128 partitions) and
    # scores contract K=64 in a PE quadrant. Otherwise qT is zero-padded per
    # head ([128, NH, T]) so score matmuls run full-K=128 against the natural
    # two-head kT chunk (the other head's rows hit zeros).
    if c.k64:
        qT_z = big.tile([128, c.MC, c.T], mmdt, tag="qT")
    else:
        qT_z = big.tile([128, c.NH, c.T], mmdt, tag="qT")
        nc.vector.memset(qT_z[:], 0.0)
    kT_sb = big.tile([128, c.MC, c.T], mmdt, tag="kT")
    v_sb = big.tile([128, c.TC, c.NH, HD1], bf16, tag="v")
    nc.vector.memset(v_sb[:, :, :, c.HD:HD1], 1.0)

    a_sb = big.tile([128, c.TC, c.NH, c.HD], bf16, tag="a")
    aT_sb = big.tile([128, c.MC, c.T], bf16, tag="aT")

    # ---- projection emitters (used as PE filler inside the attention) ----
    def emit_qk_tile(m, which, n):
        w_sb, b_sb = (wq_sb, bq_sb) if which == "q" else (wk_sb, bk_sb)
        ps = ps_mm.tile([128, 1024], f32, tag="mm")
        for d in range(W2 // 512):
            for k in range(c.KC):
                nc.tensor.matmul(
                    ps[:, d * 512:(d + 1) * 512],
                    lhsT=w_sb[:, k, m * 128:(m + 1) * 128],
                    rhs=xT_sb[:, k, n * W2 + d * 512:n * W2 + (d + 1) * 512],
                    start=(k == 0), stop=(k == c.KC - 1),
                )
        sl = slice(n * W2, (n + 1) * W2)
        if which == "k":
            nc.vector.tensor_scalar_add(
                kT_sb[:, m, sl], ps[:, 0:W2], b_sb[:, m:m + 1],
            )
        elif c.k64:
            nc.vector.tensor_scalar_add(
                qT_z[:, m, sl], ps[:, 0:W2], b_sb[:, m:m + 1],
            )
        else:
            nc.vector.tensor_scalar_add(
                qT_z[0:64, 2 * m, sl], ps[0:64, 0:W2], b_sb[0:64, m:m + 1],
            )
            nc.vector.tensor_scalar_add(
                qT_z[64:128, 2 * m + 1, sl], ps[64:128, 0:W2],
                b_sb[64:128, m:m + 1],
            )

    def emit_v_tile(tg):
        # v in normal layout, augmented with a ones column per head;
        # VG t-chunks share one psum tile.
        ps = ps_mm.tile([128, 1024], f32, tag="mm")
        for d in range(VG):
            t = tg * VG + d
            for k in range(c.KC):
                nc.tensor.matmul(
                    ps[:, d * c.NHD:(d + 1) * c.NHD],
                    lhsT=xT_sb[:, k, t * 128:(t + 1) * 128],
                    rhs=wv_sb[:, k, :],
                    start=(k == 0), stop=(k == c.KC - 1),
                )
        for d in range(VG):
            t = tg * VG + d
            nc.vector.tensor_tensor(
                out=v_sb[:, t, :, 0:c.HD],
                in0=ps[:, d * c.NHD:(d + 1) * c.NHD].rearrange(
                    "p (h d) -> p h d", d=c.HD),
                in1=bvb_sb.rearrange("p (h d) -> p h d", d=c.HD),
                op=ALU.add,
            )

    # ---- attention unit emitters ----
    def emit_scores(g, hp, fillers):
        """Score matmuls + exp for unit (g, hp); fillers (projection-tile
        thunks) are spread between score tiles to keep the PE fed while the
        ACT engine works through the exp backlog."""
        fillers = list(fillers)
        kmax = (g + 1) * c.QT if causal else c.TC
        assert kmax % 2 == 0
        tiles = [(kp, hl) for kp in range(kmax // 2) for hl in range(2)]
        etiles = {}
        nfill = len(fillers)
        done_f = 0
        for idx, (kp, hl) in enumerate(tiles):
            h = 2 * hp + hl
            ps = ps_mm.tile([128, 1024], f32, tag="mm")
            lo0 = 0
            for d in range(2):
                kc = 2 * kp + d
                off = (kc - g * c.QT) * 128 if causal else -1
                lo = max(0, off)
                if d == 0:
                    lo0 = lo
                if c.k64:
                    hs = 64 * hl
                    nc.tensor.matmul(
                        ps[:, d * 512 + lo:d * 512 + c.QW],
                        lhsT=kT_sb[hs:hs + 64, hp, kc * 128:(kc + 1) * 128],
                        rhs=qT_z[hs:hs + 64, hp,
                                 g * c.QW + lo:(g + 1) * c.QW],
                        start=True, stop=True,
                    )
                else:
                    nc.tensor.matmul(
                        ps[:, d * 512 + lo:d * 512 + c.QW],
                        lhsT=kT_sb[:, hp, kc * 128:(kc + 1) * 128],
                        rhs=qT_z[:, h, g * c.QW + lo:(g + 1) * c.QW],
                        start=True, stop=True,
                    )
                if causal:
                    if off >= 0:
                        nc.vector.tensor_tensor(
                            out=ps[:, d * 512 + off:d * 512 + off + 128],
                            in0=ps[:, d * 512 + off:d * 512 + off + 128],
                            in1=tri[:], op=ALU.add,
                        )
                elif c.mode == "bias":
                    mb = bias_pool.tile([128, c.QW], f32, tag="mb")
                    nc.sync.dma_start(
                        out=mb[:],
                        in_=maskb[kc * 128:(kc + 1) * 128,
                                  g * c.QW:(g + 1) * c.QW],
                    )
                    nc.vector.tensor_tensor(
                        out=ps[:, d * 512:d * 512 + c.QW],
                        in0=ps[:, d * 512:d * 512 + c.QW],
                        in1=mb[:], op=ALU.add,
                    )
            et = epool.tile([128, 1024], bf16, tag="E")
            # exp only the columns PV will read (cols < lo0 are fully masked
            # for both chunks; stale et/psum bytes there are never consumed)
            nc.scalar.activation(et[:, lo0:1024], ps[:, lo0:1024],
                                 AF.Exp, scale=scale)
            etiles[(hl, kp)] = et
            want = ((idx + 1) * nfill) // len(tiles)
            while done_f < want:
                fillers[done_f]()
                done_f += 1
        while done_f < nfill:
            fillers[done_f]()
            done_f += 1
        return etiles

    def emit_pv(g, hp, etiles, qt_cb=None):
        """PV matmuls + softmax normalization for unit (g, hp). QT per-q-tile
        pv accumulators share one psum tile so the reciprocals batch. For the
        last head pair, transpose a -> aT on the PE; qt_cb(j) (flush path)
        emits the out-projection for each q tile right after its transpose."""
        last = hp == NHP - 1
        for hl in range(2):
            h = 2 * hp + hl
            # QT pv accumulators padded to 128 f32 each share one psum bank
            psv = ps_pv.tile([128, c.QT, 128], f32, tag="pv")
            for j in range(c.QT):
                qt = g * c.QT + j
                kn = qt + 1 if causal else c.TC
                for kc in range(kn):
                    kp, d = divmod(kc, 2)
                    nc.tensor.matmul(
                        psv[:, j, 0:HD1],
                        lhsT=etiles[(hl, kp)][
                            :, d * 512 + j * 128:d * 512 + (j + 1) * 128],
                        rhs=v_sb[:, kc, h, :],
                        start=(kc == 0), stop=(kc == kn - 1),
                    )
            r = rpool.tile([128, c.QT], f32, tag="r")
            nc.vector.reciprocal(r[:], psv[:, :, c.HD:HD1])
            for j in range(c.QT):
                qt = g * c.QT + j
                nc.vector.tensor_scalar_mul(
                    a_sb[:, qt, h, :], psv[:, j, 0:c.HD],
                    r[:, j:j + 1],
                )
        if last:
            # PE-transpose a -> aT (head pair ci per 128-wide tile); avoids
            # the DRAM round-trip DMA transpose and its end-of-group stall
            for j in range(c.QT):
                qt = g * c.QT + j
                for ci in range(c.MC):
                    # reuse the pv psum buffers (bf16 view) for the transpose
                    pts = ps_pv.tile([128, c.QT, 128], f32, tag="pv",
                                     name="pt")
                    pt = pts.bitcast(bf16)[:, 0, 0:128]
                    nc.tensor.transpose(
                        pt, a_sb[:, qt, 2 * ci:2 * ci + 2, :], ident[:],
                    )
                    nc.vector.tensor_copy(
                        aT_sb[:, ci, qt * 128:(qt + 1) * 128], pt,
                    )
                if qt_cb is not None:
                    qt_cb(j)

    def emit_outproj_tile(g, j, flush=False):
        t = g * c.QT + j
        ps = ps_mm.tile([128, 1024], f32, tag="mm")
        for d in range(c.DM // 512):
            for ci in range(c.MC):
                nc.tensor.matmul(
                    ps[:, d * 512:(d + 1) * 512],
                    lhsT=aT_sb[:, ci, t * 128:(t + 1) * 128],
                    rhs=wo_sb[:, ci, d * 512:(d + 1) * 512],
                    start=(ci == 0), stop=(ci == c.MC - 1),
                )
        ot = ostage.tile([128, c.DM], bf16, tag="o")
        # at the flush ACT is idle: alternate DVE/ACT so casts overlap
        if flush and j % 2 == 1:
            nc.scalar.copy(ot[:], ps[:, 0:c.DM])
        else:
            nc.vector.tensor_copy(ot[:], ps[:, 0:c.DM])
        nc.sync.dma_start(
            out=out[t * 128:(t + 1) * 128, :], in_=ot[:],
        )

    def emit_outproj(g):
        for j in range(c.QT):
            emit_outproj_tile(g, j)

    # ---- schedule ----
    def qk_thunks(m, n):
        return [lambda m=m, n=n: emit_qk_tile(m, "q", n),
                lambda m=m, n=n: emit_qk_tile(m, "k", n)]

    def v_thunk(tg):
        return [lambda tg=tg: emit_v_tile(tg)]

    if causal and c.QG == 4 and NVT == 4 and NHP == 2 and NB == 2:
        # group order [1,3,2,0]: g=1 starts fast (needs only the first qk
        # block), g=0 (cheapest) last to minimize the serial tail.
        g_seq = [1, 3, 2, 0]
        prelude = [(0, 0)]
        # fillers ordered by input-DMA arrival: v tiles unblock before the
        # second-half qk blocks (which need the tail of xT)
        fill = {
            0: qk_thunks(1, 0) + v_thunk(0),
            1: v_thunk(1) + qk_thunks(0, 1),
            2: v_thunk(2) + qk_thunks(1, 1),
            3: v_thunk(3),
        }
        lag = 1
    else:
        g_seq = list(range(c.QG))
        prelude = [(m, n) for m in range(NHP) for n in range(NB)]
        fill = {0: [t for tg in range(NVT) for t in v_thunk(tg)]}
        lag = 0

    for m, n in prelude:
        emit_qk_tile(m, "q", n)
        emit_qk_tile(m, "k", n)

    units = [(g, hp) for g in g_seq for hp in range(NHP)]
    pending_pv = []     # (g, hp, etiles) awaiting PV emission
    ready_op = []       # groups whose aT transposes were issued last step
    for i, (g, hp) in enumerate(units):
        etiles = emit_scores(g, hp, fill.get(i, []))
        # withhold outproj on the last iteration: it fills the PE's wait for
        # the final unit's exps during the flush instead
        if i < len(units) - 1:
            while ready_op:
                emit_outproj(ready_op.pop(0))
        pending_pv.append((g, hp, etiles))
        if len(pending_pv) > lag:
            pg, php, pet = pending_pv.pop(0)
            emit_pv(pg, php, pet)
            if php == NHP - 1:
                ready_op.append(pg)
    for pg, php, pet in pending_pv:
        while ready_op:
            emit_outproj(ready_op.pop(0))
        if php == NHP - 1:
            # flush: out-projection of each q tile rides right behind its
            # aT transpose so the final casts/DMAs start as early as possible
            emit_pv(pg, php, pet,
                    qt_cb=lambda j, pg=pg: emit_outproj_tile(pg, j, flush=True))
        else:
            emit_pv(pg, php, pet)
    while ready_op:
        emit_outproj(ready_op.pop(0))


# ---------------------------------------------------------------------------
# host side
# ---------------------------------------------------------------------------

_CACHE: dict = {}


def _get_program(cfg: Cfg):
    key = cfg
    if key not in _CACHE:
        _CACHE[key] = build_program(cfg)
    return _CACHE[key]


def _mask_mode(mask: np.ndarray, T: int) -> str:
    m = (np.asarray(mask).reshape(T, T) != 0)
    if m.all():
        return "full"
    if np.array_equal(m, np.tril(np.ones((T, T), dtype=bool))):
        return "causal"
    return "bias"


def make_in_maps(cfg: Cfg, x, W_qkv, b_qkv, W_out, mask=None):
    """Slice full inputs into the 8 per-core input dicts."""
    c = cfg
    npmm = c.npmm
    B = x.shape[0]
    n_hg = N_CORES // B                      # head groups per batch
    in_maps = []
    maskb = None
    if c.mode == "bias":
        m = (np.asarray(mask).reshape(c.T, c.T) != 0)
        maskb = np.where(m, np.float32(0), np.float32(NEG)).T.copy()
    for core in range(N_CORES):
        b, hg = divmod(core, n_hg)
        col0 = hg * c.NHD
        xT = np.ascontiguousarray(x[b].T).astype(npmm)
        wq_ = np.ascontiguousarray(W_qkv[:, 0 * c.DM + col0:0 * c.DM + col0 + c.NHD]).astype(npmm)
        wk_ = np.ascontiguousarray(W_qkv[:, 1 * c.DM + col0:1 * c.DM + col0 + c.NHD]).astype(npmm)
        wv_ = np.ascontiguousarray(W_qkv[:, 2 * c.DM + col0:2 * c.DM + col0 + c.NHD]).astype(npmm)
        bq_ = np.ascontiguousarray(
            b_qkv[0 * c.DM + col0:0 * c.DM + col0 + c.NHD].reshape(c.MC, 128).T
        ).astype(np.float32)
        bk_ = np.ascontiguousarray(
            b_qkv[1 * c.DM + col0:1 * c.DM + col0 + c.NHD].reshape(c.MC, 128).T
        ).astype(np.float32)
        bv_ = b_qkv[2 * c.DM + col0:2 * c.DM + col0 + c.NHD].astype(np.float32)
        bvb_ = np.ascontiguousarray(np.broadcast_to(bv_, (128, c.NHD)))
        biases_ = np.ascontiguousarray(
            np.concatenate([bq_, bk_, bvb_], axis=1))
        wo_ = np.ascontiguousarray(W_out[col0:col0 + c.NHD, :]).astype(npmm)
        im = dict(xT=xT, wq=wq_, wk=wk_, wv=wv_, biases=biases_,
                  wo=wo_, ident=np.eye(128, dtype=ml_dtypes.bfloat16))
        if c.mode == "bias":
            im["maskb"] = maskb
        in_maps.append(im)
    return in_maps


def run_sharded(cfg: Cfg, x, W_qkv, b_qkv, W_out, b_out, mask=None, **kw):
    """Run the SPMD program on 8 cores and assemble the full output."""
    nc, _names = _get_program(cfg)
    in_maps = make_in_maps(cfg, x, W_qkv, b_qkv, W_out, mask)
    res = bass_utils.run_bass_kernel_spmd(
        nc, in_maps, core_ids=list(range(N_CORES)), **kw,
    )
    outs = [np.asarray(r["out"]).astype(np.float32) for r in res.results]
    B = x.shape[0]
    n_hg = N_CORES // B
    y = np.stack([
        np.sum(outs[b * n_hg:(b + 1) * n_hg], axis=0) for b in range(B)
    ]) + b_out.astype(np.float32)
    return y.astype(np.float32), res


def kernel(x, W_qkv, b_qkv, W_out, b_out, mask):
    x = np.asarray(x, dtype=np.float32)
    W_qkv = np.asarray(W_qkv, dtype=np.float32)
    b_qkv = np.asarray(b_qkv, dtype=np.float32)
    W_out = np.asarray(W_out, dtype=np.float32)
    b_out = np.asarray(b_out, dtype=np.float32)
    B, T, DM = x.shape
    mode = _mask_mode(mask, T)
    cfg = Cfg(T=T, DM=DM, mode=mode, mm=os.environ.get("MHA_MM_DT", "bf16"))
    y, _ = run_sharded(cfg, x, W_qkv, b_qkv, W_out, b_out, mask)
    return y


# revision 44
# speedup vs baseline: 1.0716x; 1.0716x over previous
"""Multi-head attention (B=2, T=2048, d_model=1024, H=16, hd=64) on 8 Trainium2
NeuronCores.

Sharding: the 32 (batch, head) attention units are split as 4 consecutive heads
of one batch per core (core c -> batch c//4, heads 4*(c%4) .. 4*(c%4)+3). Each
core computes its own QKV projection slice, causal attention for its heads, and
a partial out-projection (its 256 rows of W_out). The host sums the 4 partials
per batch and adds b_out.

Device-side layout (everything flows transposed so no on-chip transposes are
needed until the attention output):
  qT/kT [hd, T]  <- lhsT=W slice, rhs=xT
  v     [T, hd]  (+ ones column for the row-sum trick)
  sT    [k, q]   <- lhsT=kT chunk, rhs=qT          (psum, fp32)
  E     [k, q]   <- exp(sT * 1/sqrt(hd)) on ScalarE (bf16)
  pv    [q, hd+1]<- lhsT=E chunk, rhs=[v|1]        (col hd = row sum)
  a     [q, hd]  = pv[:, :hd] * (1/pv[:, hd])      (per-partition scalar)
  aT    [hd, T]  via DRAM round-trip DMA transpose
  out  += aT.T @ W_out slice                        (partial, fp32)

Schedule: the attention phase is ACT-bound (exp is ~45% of all engine work),
so emission is software-pipelined per (q-group, head-pair) unit: scores+exp of
unit i are emitted together with the PV matmuls of unit i-1 and the out-
projection of earlier-finalized q-groups, with the remaining QKV-projection and
v tiles spread between score tiles as PE filler. Group order [1,3,2,0] puts a
cheap group last to shrink the serial tail. Diagonal q-blocks are trimmed from
both the score matmuls and the exp.
"""

import math
import os
from contextlib import ExitStack
from dataclasses import dataclass

import numpy as np
import ml_dtypes

import concourse.bass as bass
import concourse.tile as tile
from concourse import bacc, mybir
from concourse import bass_utils

AF = mybir.ActivationFunctionType
ALU = mybir.AluOpType
DT = mybir.dt

N_CORES = 8
NEG = -1e9


@dataclass(frozen=True)
class Cfg:
    T: int = 2048        # sequence length
    DM: int = 1024       # d_model
    HD: int = 64         # head dim
    NH: int = 4          # heads per core
    mode: str = "causal"  # "causal" | "full" | "bias"
    mm: str = "bf16"     # matmul operand dtype: "bf16" | "f32r" | "f32"
    # K=64 score matmuls (PE quadrant) instead of zero-padded K=128.
    # Measured WORSE (throttle_active 70us vs 40us, util limit 0.71 vs 0.77):
    # the DVFS clamps harder on half-utilized PE arrays. Keep off.
    k64: bool = os.environ.get("MHA_K64", "0") == "1"

    @property
    def NHD(self):
        return self.NH * self.HD          # qkv slice width per core

    @property
    def KC(self):
        return self.DM // 128             # contraction chunks for projections

    @property
    def MC(self):
        return self.NHD // 128            # qT/kT partition chunks

    @property
    def TC(self):
        return self.T // 128              # t chunks

    @property
    def QW(self):
        return min(512, self.T)           # q group width

    @property
    def QG(self):
        return self.T // self.QW

    @property
    def QT(self):
        return self.QW // 128             # q tiles per group

    @property
    def mmdt(self):
        return {"bf16": DT.bfloat16, "f32r": DT.float32r, "f32": DT.float32}[self.mm]

    @property
    def npmm(self):
        return ml_dtypes.bfloat16 if self.mm == "bf16" else np.float32


def build_program(cfg: Cfg):
    """Build + compile the SPMD single-core program. Returns (nc, input_names)."""
    c = cfg
    assert c.DM % 128 == 0 and c.NHD % 128 == 0 and c.T % 512 == 0
    nc = bacc.Bacc("TRN2", target_bir_lowering=False, debug=False,
                   num_devices=N_CORES)
    f32 = DT.float32
    mmdt = c.mmdt

    xT = nc.dram_tensor("xT", [c.DM, c.T], mmdt, kind="ExternalInput").ap()
    wq = nc.dram_tensor("wq", [c.DM, c.NHD], mmdt, kind="ExternalInput").ap()
    wk = nc.dram_tensor("wk", [c.DM, c.NHD], mmdt, kind="ExternalInput").ap()
    wv = nc.dram_tensor("wv", [c.DM, c.NHD], mmdt, kind="ExternalInput").ap()
    # bq | bk | bvb packed: tiny strided DMAs cost ~2.6us of descriptor time
    biases = nc.dram_tensor("biases", [128, 2 * c.MC + c.NHD], f32,
                            kind="ExternalInput").ap()
    wo = nc.dram_tensor("wo", [c.NHD, c.DM], mmdt, kind="ExternalInput").ap()
    identd = nc.dram_tensor("ident", [128, 128], DT.bfloat16,
                            kind="ExternalInput").ap()
    maskb = None
    if c.mode == "bias":
        # additive bias, transposed: maskb[k, q]
        maskb = nc.dram_tensor("maskb", [c.T, c.T], f32, kind="ExternalInput").ap()
    # bf16 partials: host upconverts and sums in f32
    out = nc.dram_tensor("out", [c.T, c.DM], DT.bfloat16,
                         kind="ExternalOutput").ap()

    with tile.TileContext(nc) as tc, ExitStack() as ctx:
        _body(ctx, tc, c, xT, wq, wk, wv, biases, wo, identd, maskb, out)
    nc.compile()
    names = ["xT", "wq", "wk", "wv", "biases", "wo", "ident"]
    if c.mode == "bias":
        names.append("maskb")
    return nc, names


def _body(ctx, tc, c: Cfg, xT, wq, wk, wv, biases, wo, identd, maskb, out):
    nc = tc.nc
    f32 = DT.float32
    bf16 = DT.bfloat16
    mmdt = c.mmdt
    causal = c.mode == "causal"
    scale = 1.0 / math.sqrt(c.HD)
    NHP = c.NH // 2                      # head pairs per core
    HD1 = c.HD + 1
    W2 = min(1024, c.T)                  # qk projection block width
    NB = c.T // W2                       # qk projection blocks
    VG = min(c.TC, max(1, 1024 // c.NHD))  # t-chunks per v psum tile
    NVT = c.TC // VG

    const = ctx.enter_context(tc.tile_pool(name="const", bufs=1))
    big = ctx.enter_context(tc.tile_pool(name="big", bufs=1))
    # lag-1 pipeline: E tiles of two consecutive units are alive at once
    # (worst case 2 * kmax_max/2 * 2 = 32 for causal T=2048)
    epool = ctx.enter_context(
        tc.tile_pool(name="E", bufs=(8 * c.QT if causal else c.TC + 4)))
    rpool = ctx.enter_context(tc.tile_pool(name="r", bufs=8))
    # PSUM: 3 x [128,1024] (6 banks) + 2 x [128,260] (2 banks) = 8 banks
    ps_mm = ctx.enter_context(tc.tile_pool(name="psmm", bufs=3, space="PSUM"))
    ps_pv = ctx.enter_context(tc.tile_pool(name="pspv", bufs=2, space="PSUM"))
    ostage = ctx.enter_context(tc.tile_pool(name="ostage", bufs=3))
    bias_pool = None
    if c.mode == "bias":
        bias_pool = ctx.enter_context(tc.tile_pool(name="maskb", bufs=4))

    # ---- input DMAs: qk weights first, then xT in pieces (the first score
    # block only needs t<512), with everything ordered by first use so the
    # sync queue's ~0.8us/descriptor serialization doesn't delay the PE ----
    w_sbs = {}
    for nm, w in (("wq", wq), ("wk", wk), ("wv", wv)):
        w_sbs[nm] = big.tile([128, c.KC, c.NHD], mmdt, tag=nm, name=nm)
    wo_sb = big.tile([128, c.MC, c.DM], mmdt, tag="wo")
    xT_sb = big.tile([128, c.KC, c.T], mmdt, tag="xT")
    xTd = xT.rearrange("(c p) t -> p c t", p=128)
    bias_sb = const.tile([128, 2 * c.MC + c.NHD], f32, tag="bias")
    bq_sb = bias_sb[:, 0:c.MC]
    bk_sb = bias_sb[:, c.MC:2 * c.MC]
    bvb_sb = bias_sb[:, 2 * c.MC:2 * c.MC + c.NHD]

    wqd = wq.rearrange("(c p) n -> p c n", p=128)
    wkd = wk.rearrange("(c p) n -> p c n", p=128)
    # sync queue: exactly what the prelude + early score units need, in
    # first-use order (the first tile needs wq/wk m=0 halves + xT[:, :, 0:W2])
    nc.sync.dma_start(out=w_sbs["wq"][:, :, 0:128], in_=wqd[:, :, 0:128])
    nc.sync.dma_start(out=w_sbs["wk"][:, :, 0:128], in_=wkd[:, :, 0:128])
    xcuts = ([0, c.T // 4, c.T // 2, 3 * c.T // 4, c.T]
             if c.T >= 2048 else [0, c.T])
    for x0, x1 in zip(xcuts[:2], xcuts[1:3]):
        nc.sync.dma_start(out=xT_sb[:, :, x0:x1], in_=xTd[:, :, x0:x1])
    nc.sync.dma_start(out=w_sbs["wq"][:, :, 128:c.NHD],
                      in_=wqd[:, :, 128:c.NHD])
    nc.sync.dma_start(out=w_sbs["wk"][:, :, 128:c.NHD],
                      in_=wkd[:, :, 128:c.NHD])
    nc.sync.dma_start(out=w_sbs["wv"][:],
                      in_=wv.rearrange("(c p) n -> p c n", p=128))
    nc.sync.dma_start(out=bias_sb[:], in_=biases)
    for x0, x1 in zip(xcuts[2:], xcuts[3:]):
        nc.sync.dma_start(out=xT_sb[:, :, x0:x1], in_=xTd[:, :, x0:x1])
    nc.sync.dma_start(out=wo_sb[:],
                      in_=wo.rearrange("(c p) n -> p c n", p=128))
    wq_sb, wk_sb, wv_sb = w_sbs["wq"], w_sbs["wk"], w_sbs["wv"]

    # causal mask block for diagonal tiles: tri[k, j] = 0 if j >= k else NEG
    tri = const.tile([128, 128], f32, tag="tri")
    nc.gpsimd.memset(tri[:], 0.0)
    nc.gpsimd.affine_select(
        out=tri[:], in_=tri[:],
        compare_op=ALU.is_ge, fill=NEG,
        base=0, channel_multiplier=-1, pattern=[[1, 128]],
    )
    # identity (bf16) for PE transposes of the attention output (host input)
    ident = const.tile([128, 128], bf16, tag="ident")
    nc.sync.dma_start(out=ident[:], in_=identd)

    # k64: qT packed like kT ([128, MC, T], head pair per 128 partitions) and
    # scores contract K=64 in a PE quadrant. Otherwise qT is zero-padded per
    # head ([128, NH, T]) so score matmuls run full-K=128 against the natural
    # two-head kT chunk (the other head's rows hit zeros).
    if c.k64:
        qT_z = big.tile([128, c.MC, c.T], mmdt, tag="qT")
    else:
        qT_z = big.tile([128, c.NH, c.T], mmdt, tag="qT")
        nc.vector.memset(qT_z[:], 0.0)
    kT_sb = big.tile([128, c.MC, c.T], mmdt, tag="kT")
    v_sb = big.tile([128, c.TC, c.NH, HD1], bf16, tag="v")
    nc.vector.memset(v_sb[:, :, :, c.HD:HD1], 1.0)

    a_sb = big.tile([128, c.TC, c.NH, c.HD], bf16, tag="a")
    aT_sb = big.tile([128, c.MC, c.T], bf16, tag="aT")

    # ---- projection emitters (used as PE filler inside the attention) ----
    def emit_qk_tile(m, which, n):
        w_sb, b_sb = (wq_sb, bq_sb) if which == "q" else (wk_sb, bk_sb)
        ps = ps_mm.tile([128, 1024], f32, tag="mm")
        for d in range(W2 // 512):
            for k in range(c.KC):
                nc.tensor.matmul(
                    ps[:, d * 512:(d + 1) * 512],
                    lhsT=w_sb[:, k, m * 128:(m + 1) * 128],
                    rhs=xT_sb[:, k, n * W2 + d * 512:n * W2 + (d + 1) * 512],
                    start=(k == 0), stop=(k == c.KC - 1),
                )
        sl = slice(n * W2, (n + 1) * W2)
        if which == "k":
            nc.vector.tensor_scalar_add(
                kT_sb[:, m, sl], ps[:, 0:W2], b_sb[:, m:m + 1],
            )
        elif c.k64:
            nc.vector.tensor_scalar_add(
                qT_z[:, m, sl], ps[:, 0:W2], b_sb[:, m:m + 1],
            )
        else:
            nc.vector.tensor_scalar_add(
                qT_z[0:64, 2 * m, sl], ps[0:64, 0:W2], b_sb[0:64, m:m + 1],
            )
            nc.vector.tensor_scalar_add(
                qT_z[64:128, 2 * m + 1, sl], ps[64:128, 0:W2],
                b_sb[64:128, m:m + 1],
            )

    def emit_v_tile(tg):
        # v in normal layout, augmented with a ones column per head;
        # VG t-chunks share one psum tile.
        ps = ps_mm.tile([128, 1024], f32, tag="mm")
        for d in range(VG):
            t = tg * VG + d
            for k in range(c.KC):
                nc.tensor.matmul(
                    ps[:, d * c.NHD:(d + 1) * c.NHD],
                    lhsT=xT_sb[:, k, t * 128:(t + 1) * 128],
                    rhs=wv_sb[:, k, :],
                    start=(k == 0), stop=(k == c.KC - 1),
                )
        for d in range(VG):
            t = tg * VG + d
            nc.vector.tensor_tensor(
                out=v_sb[:, t, :, 0:c.HD],
                in0=ps[:, d * c.NHD:(d + 1) * c.NHD].rearrange(
                    "p (h d) -> p h d", d=c.HD),
                in1=bvb_sb.rearrange("p (h d) -> p h d", d=c.HD),
                op=ALU.add,
            )

    # ---- attention unit emitters ----
    def emit_scores(g, hp, fillers):
        """Score matmuls + exp for unit (g, hp); fillers (projection-tile
        thunks) are spread between score tiles to keep the PE fed while the
        ACT engine works through the exp backlog."""
        fillers = list(fillers)
        kmax = (g + 1) * c.QT if causal else c.TC
        assert kmax % 2 == 0
        tiles = [(kp, hl) for kp in range(kmax // 2) for hl in range(2)]
        etiles = {}
        nfill = len(fillers)
        done_f = 0
        for idx, (kp, hl) in enumerate(tiles):
            h = 2 * hp + hl
            ps = ps_mm.tile([128, 1024], f32, tag="mm")
            lo0 = 0
            for d in range(2):
                kc = 2 * kp + d
                off = (kc - g * c.QT) * 128 if causal else -1
                lo = max(0, off)
                if d == 0:
                    lo0 = lo
                if c.k64:
                    hs = 64 * hl
                    nc.tensor.matmul(
                        ps[:, d * 512 + lo:d * 512 + c.QW],
                        lhsT=kT_sb[hs:hs + 64, hp, kc * 128:(kc + 1) * 128],
                        rhs=qT_z[hs:hs + 64, hp,
                                 g * c.QW + lo:(g + 1) * c.QW],
                        start=True, stop=True,
                    )
                else:
                    nc.tensor.matmul(
                        ps[:, d * 512 + lo:d * 512 + c.QW],
                        lhsT=kT_sb[:, hp, kc * 128:(kc + 1) * 128],
                        rhs=qT_z[:, h, g * c.QW + lo:(g + 1) * c.QW],
                        start=True, stop=True,
                    )
                if causal:
                    if off >= 0:
                        nc.vector.tensor_tensor(
                            out=ps[:, d * 512 + off:d * 512 + off + 128],
                            in0=ps[:, d * 512 + off:d * 512 + off + 128],
                            in1=tri[:], op=ALU.add,
                        )
                elif c.mode == "bias":
                    mb = bias_pool.tile([128, c.QW], f32, tag="mb")
                    nc.sync.dma_start(
                        out=mb[:],
                        in_=maskb[kc * 128:(kc + 1) * 128,
                                  g * c.QW:(g + 1) * c.QW],
                    )
                    nc.vector.tensor_tensor(
                        out=ps[:, d * 512:d * 512 + c.QW],
                        in0=ps[:, d * 512:d * 512 + c.QW],
                        in1=mb[:], op=ALU.add,
                    )
            et = epool.tile([128, 1024], bf16, tag="E")
            # exp only the columns PV will read (cols < lo0 are fully masked
            # for both chunks; stale et/psum bytes there are never consumed)
            nc.scalar.activation(et[:, lo0:1024], ps[:, lo0:1024],
                                 AF.Exp, scale=scale)
            etiles[(hl, kp)] = et
            want = ((idx + 1) * nfill) // len(tiles)
            while done_f < want:
                fillers[done_f]()
                done_f += 1
        while done_f < nfill:
            fillers[done_f]()
            done_f += 1
        return etiles

    def emit_pv(g, hp, etiles, qt_cb=None):
        """PV matmuls + softmax normalization for unit (g, hp). QT per-q-tile
        pv accumulators share one psum tile so the reciprocals batch. For the
        last head pair, transpose a -> aT on the PE; qt_cb(j) (flush path)
        emits the out-projection for each q tile right after its transpose."""
        last = hp == NHP - 1
        for hl in range(2):
            h = 2 * hp + hl
            # QT pv accumulators padded to 128 f32 each share one psum bank
            psv = ps_pv.tile([128, c.QT, 128], f32, tag="pv")
            for j in range(c.QT):
                qt = g * c.QT + j
                kn = qt + 1 if causal else c.TC
                for kc in range(kn):
                    kp, d = divmod(kc, 2)
                    nc.tensor.matmul(
                        psv[:, j, 0:HD1],
                        lhsT=etiles[(hl, kp)][
                            :, d * 512 + j * 128:d * 512 + (j + 1) * 128],
                        rhs=v_sb[:, kc, h, :],
                        start=(kc == 0), stop=(kc == kn - 1),
                    )
            r = rpool.tile([128, c.QT], f32, tag="r")
            nc.vector.reciprocal(r[:], psv[:, :, c.HD:HD1])
            for j in range(c.QT):
                qt = g * c.QT + j
                nc.vector.tensor_scalar_mul(
                    a_sb[:, qt, h, :], psv[:, j, 0:c.HD],
                    r[:, j:j + 1],
                )
        if last:
            # PE-transpose a -> aT (head pair ci per 128-wide tile); avoids
            # the DRAM round-trip DMA transpose and its end-of-group stall
            for j in range(c.QT):
                qt = g * c.QT + j
                for ci in range(c.MC):
                    # reuse the pv psum buffers (bf16 view) for the transpose
                    pts = ps_pv.tile([128, c.QT, 128], f32, tag="pv",
                                     name="pt")
                    pt = pts.bitcast(bf16)[:, 0, 0:128]
                    nc.tensor.transpose(
                        pt, a_sb[:, qt, 2 * ci:2 * ci + 2, :], ident[:],
                    )
                    nc.vector.tensor_copy(
                        aT_sb[:, ci, qt * 128:(qt + 1) * 128], pt,
                    )
                if qt_cb is not None:
                    qt_cb(j)

    def emit_outproj_tile(g, j, flush=False):
        t = g * c.QT + j
        ps = ps_mm.tile([128, 1024], f32, tag="mm")
        for d in range(c.DM // 512):
            for ci in range(c.MC):
                nc.tensor.matmul(
                    ps[:, d * 512:(d + 1) * 512],
                    lhsT=aT_sb[:, ci, t * 128:(t + 1) * 128],
                    rhs=wo_sb[:, ci, d * 512:(d + 1) * 512],
                    start=(ci == 0), stop=(ci == c.MC - 1),
                )
        ot = ostage.tile([128, c.DM], bf16, tag="o")
        # at the flush ACT is idle: alternate DVE/ACT so casts overlap
        if flush and j % 2 == 1:
            nc.scalar.copy(ot[:], ps[:, 0:c.DM])
        else:
            nc.vector.tensor_copy(ot[:], ps[:, 0:c.DM])
        nc.sync.dma_start(
            out=out[t * 128:(t + 1) * 128, :], in_=ot[:],
        )

    def emit_outproj(g):
        for j in range(c.QT):
            emit_outproj_tile(g, j)

    # ---- schedule ----
    def qk_thunks(m, n):
        return [lambda m=m, n=n: emit_qk_tile(m, "q", n),
                lambda m=m, n=n: emit_qk_tile(m, "k", n)]

    def v_thunk(tg):
        return [lambda tg=tg: emit_v_tile(tg)]

    if causal and c.QG == 4 and NVT == 4 and NHP == 2 and NB == 2:
        # group order [1,3,2,0]: g=1 starts fast (needs only the first qk
        # block), g=0 (cheapest) last to minimize the serial tail.
        g_seq = [1, 3, 2, 0]
        prelude = [(0, 0)]
        # fillers ordered by input-DMA arrival: v tiles unblock before the
        # second-half qk blocks (which need the tail of xT)
        fill = {
            0: qk_thunks(1, 0) + v_thunk(0),
            1: v_thunk(1) + qk_thunks(0, 1),
            2: v_thunk(2) + qk_thunks(1, 1),
            3: v_thunk(3),
        }
        lag = 1
    else:
        g_seq = list(range(c.QG))
        prelude = [(m, n) for m in range(NHP) for n in range(NB)]
        fill = {0: [t for tg in range(NVT) for t in v_thunk(tg)]}
        lag = 0

    for m, n in prelude:
        emit_qk_tile(m, "q", n)
        emit_qk_tile(m, "k", n)

    units = [(g, hp) for g in g_seq for hp in range(NHP)]
    pending_pv = []     # (g, hp, etiles) awaiting PV emission
    ready_op = []       # groups whose aT transposes were issued last step
    for i, (g, hp) in enumerate(units):
        etiles = emit_scores(g, hp, fill.get(i, []))
        # withhold outproj on the last iteration: it fills the PE's wait for
        # the final unit's exps during the flush instead
        if i < len(units) - 1:
            while ready_op:
                emit_outproj(ready_op.pop(0))
        pending_pv.append((g, hp, etiles))
        if len(pending_pv) > lag:
            pg, php, pet = pending_pv.pop(0)
            emit_pv(pg, php, pet)
            if php == NHP - 1:
                ready_op.append(pg)
    for pg, php, pet in pending_pv:
        while ready_op:
            emit_outproj(ready_op.pop(0))
        if php == NHP - 1:
            # flush: out-projection of each q tile rides right behind its
            # aT transpose so the final casts/DMAs start as early as possible
            emit_pv(pg, php, pet,
                    qt_cb=lambda j, pg=pg: emit_outproj_tile(pg, j, flush=True))
        else:
            emit_pv(pg, php, pet)
    while ready_op:
        emit_outproj(ready_op.pop(0))


# ---------------------------------------------------------------------------
# host side
# ---------------------------------------------------------------------------

_CACHE: dict = {}


def _get_program(cfg: Cfg):
    key = cfg
    if key not in _CACHE:
        _CACHE[key] = build_program(cfg)
    return _CACHE[key]


def _mask_mode(mask: np.ndarray, T: int) -> str:
    m = (np.asarray(mask).reshape(T, T) != 0)
    if m.all():
        return "full"
    if np.array_equal(m, np.tril(np.ones((T, T), dtype=bool))):
        return "causal"
    return "bias"


def make_in_maps(cfg: Cfg, x, W_qkv, b_qkv, W_out, mask=None):
    """Slice full inputs into the 8 per-core input dicts."""
    c = cfg
    npmm = c.npmm
    B = x.shape[0]
    n_hg = N_CORES // B                      # head groups per batch
    in_maps = []
    maskb = None
    if c.mode == "bias":
        m = (np.asarray(mask).reshape(c.T, c.T) != 0)
        maskb = np.where(m, np.float32(0), np.float32(NEG)).T.copy()
    for core in range(N_CORES):
        b, hg = divmod(core, n_hg)
        col0 = hg * c.NHD
        xT = np.ascontiguousarray(x[b].T).astype(npmm)
        wq_ = np.ascontiguousarray(W_qkv[:, 0 * c.DM + col0:0 * c.DM + col0 + c.NHD]).astype(npmm)
        wk_ = np.ascontiguousarray(W_qkv[:, 1 * c.DM + col0:1 * c.DM + col0 + c.NHD]).astype(npmm)
        wv_ = np.ascontiguousarray(W_qkv[:, 2 * c.DM + col0:2 * c.DM + col0 + c.NHD]).astype(npmm)
        bq_ = np.ascontiguousarray(
            b_qkv[0 * c.DM + col0:0 * c.DM + col0 + c.NHD].reshape(c.MC, 128).T
        ).astype(np.float32)
        bk_ = np.ascontiguousarray(
            b_qkv[1 * c.DM + col0:1 * c.DM + col0 + c.NHD].reshape(c.MC, 128).T
        ).astype(np.float32)
        bv_ = b_qkv[2 * c.DM + col0:2 * c.DM + col0 + c.NHD].astype(np.float32)
        bvb_ = np.ascontiguousarray(np.broadcast_to(bv_, (128, c.NHD)))
        biases_ = np.ascontiguousarray(
            np.concatenate([bq_, bk_, bvb_], axis=1))
        wo_ = np.ascontiguousarray(W_out[col0:col0 + c.NHD, :]).astype(npmm)
        im = dict(xT=xT, wq=wq_, wk=wk_, wv=wv_, biases=biases_,
                  wo=wo_, ident=np.eye(128, dtype=ml_dtypes.bfloat16))
        if c.mode == "bias":
            im["maskb"] = maskb
        in_maps.append(im)
    return in_maps


def run_sharded(cfg: Cfg, x, W_qkv, b_qkv, W_out, b_out, mask=None, **kw):
    """Run the SPMD program on 8 cores and assemble the full output."""
    nc, _names = _get_program(cfg)
    in_maps = make_in_maps(cfg, x, W_qkv, b_qkv, W_out, mask)
    res = bass_utils.run_bass_kernel_spmd(
        nc, in_maps, core_ids=list(range(N_CORES)), **kw,
    )
    outs = [np.asarray(r["out"]).astype(np.float32) for r in res.results]
    B = x.shape[0]
    n_hg = N_CORES // B
    y = np.stack([
        np.sum(outs[b * n_hg:(b + 1) * n_hg], axis=0) for b in range(B)
    ]) + b_out.astype(np.float32)
    return y.astype(np.float32), res


def kernel(x, W_qkv, b_qkv, W_out, b_out, mask):
    x = np.asarray(x, dtype=np.float32)
    W_qkv = np.asarray(W_qkv, dtype=np.float32)
    b_qkv = np.asarray(b_qkv, dtype=np.float32)
    W_out = np.asarray(W_out, dtype=np.float32)
    b_out = np.asarray(b_out, dtype=np.float32)
    B, T, DM = x.shape
    mode = _mask_mode(mask, T)
    cfg = Cfg(T=T, DM=DM, mode=mode, mm=os.environ.get("MHA_MM_DT", "bf16"))
    y, _ = run_sharded(cfg, x, W_qkv, b_qkv, W_out, b_out, mask)
    return y


# revision 46
# speedup vs baseline: 1.1177x; 1.0430x over previous
"""Multi-head attention (B=2, T=2048, d_model=1024, H=16, hd=64) on 8 Trainium2
NeuronCores.

Sharding: the 32 (batch, head) attention units are split as 4 consecutive heads
of one batch per core (core c -> batch c//4, heads 4*(c%4) .. 4*(c%4)+3). Each
core computes its own QKV projection slice, causal attention for its heads, and
a partial out-projection (its 256 rows of W_out). The host sums the 4 partials
per batch and adds b_out.

Device-side layout (everything flows transposed so no on-chip transposes are
needed until the attention output):
  qT/kT [hd, T]  <- lhsT=W slice, rhs=xT
  v     [T, hd]  (+ ones column for the row-sum trick)
  sT    [k, q]   <- lhsT=kT chunk, rhs=qT          (psum, fp32)
  E     [k, q]   <- exp(sT * 1/sqrt(hd)) on ScalarE (bf16)
  pv    [q, hd+1]<- lhsT=E chunk, rhs=[v|1]        (col hd = row sum)
  a     [q, hd]  = pv[:, :hd] * (1/pv[:, hd])      (per-partition scalar)
  aT    [hd, T]  via DRAM round-trip DMA transpose
  out  += aT.T @ W_out slice                        (partial, fp32)

Schedule: the attention phase is ACT-bound (exp is ~45% of all engine work),
so emission is software-pipelined per (q-group, head-pair) unit: scores+exp of
unit i are emitted together with the PV matmuls of unit i-1 and the out-
projection of earlier-finalized q-groups, with the remaining QKV-projection and
v tiles spread between score tiles as PE filler. Group order [1,3,2,0] puts a
cheap group last to shrink the serial tail. Diagonal q-blocks are trimmed from
both the score matmuls and the exp.
"""

import math
import os
from contextlib import ExitStack
from dataclasses import dataclass

import numpy as np
import ml_dtypes

import concourse.bass as bass
import concourse.tile as tile
from concourse import bacc, mybir
from concourse import bass_utils

AF = mybir.ActivationFunctionType
ALU = mybir.AluOpType
DT = mybir.dt

N_CORES = 8
NEG = -1e9


@dataclass(frozen=True)
class Cfg:
    T: int = 2048        # sequence length
    DM: int = 1024       # d_model
    HD: int = 64         # head dim
    NH: int = 4          # heads per core
    mode: str = "causal"  # "causal" | "full" | "bias"
    mm: str = "bf16"     # matmul operand dtype: "bf16" | "f32r" | "f32"
    # K=64 score matmuls (PE quadrant) instead of zero-padded K=128.
    # Measured WORSE (throttle_active 70us vs 40us, util limit 0.71 vs 0.77):
    # the DVFS clamps harder on half-utilized PE arrays. Keep off.
    k64: bool = os.environ.get("MHA_K64", "0") == "1"

    @property
    def NHD(self):
        return self.NH * self.HD          # qkv slice width per core

    @property
    def KC(self):
        return self.DM // 128             # contraction chunks for projections

    @property
    def MC(self):
        return self.NHD // 128            # qT/kT partition chunks

    @property
    def TC(self):
        return self.T // 128              # t chunks

    @property
    def QW(self):
        return min(512, self.T)           # q group width

    @property
    def QG(self):
        return self.T // self.QW

    @property
    def QT(self):
        return self.QW // 128             # q tiles per group

    @property
    def mmdt(self):
        return {"bf16": DT.bfloat16, "f32r": DT.float32r, "f32": DT.float32}[self.mm]

    @property
    def npmm(self):
        return ml_dtypes.bfloat16 if self.mm == "bf16" else np.float32


def build_program(cfg: Cfg):
    """Build + compile the SPMD single-core program. Returns (nc, input_names)."""
    c = cfg
    assert c.DM % 128 == 0 and c.NHD % 128 == 0 and c.T % 512 == 0
    nc = bacc.Bacc("TRN2", target_bir_lowering=False, debug=False,
                   num_devices=N_CORES)
    f32 = DT.float32
    mmdt = c.mmdt

    xT = nc.dram_tensor("xT", [c.DM, c.T], mmdt, kind="ExternalInput").ap()
    wq = nc.dram_tensor("wq", [c.DM, c.NHD], mmdt, kind="ExternalInput").ap()
    wk = nc.dram_tensor("wk", [c.DM, c.NHD], mmdt, kind="ExternalInput").ap()
    wv = nc.dram_tensor("wv", [c.DM, c.NHD], mmdt, kind="ExternalInput").ap()
    # bq | bk | bvb packed: tiny strided DMAs cost ~2.6us of descriptor time
    biases = nc.dram_tensor("biases", [128, 2 * c.MC + c.NHD], f32,
                            kind="ExternalInput").ap()
    wo = nc.dram_tensor("wo", [c.NHD, c.DM], mmdt, kind="ExternalInput").ap()
    identd = nc.dram_tensor("ident", [128, 128], DT.bfloat16,
                            kind="ExternalInput").ap()
    maskb = None
    if c.mode == "bias":
        # additive bias, transposed: maskb[k, q]
        maskb = nc.dram_tensor("maskb", [c.T, c.T], f32, kind="ExternalInput").ap()
    # bf16 partials: host upconverts and sums in f32
    out = nc.dram_tensor("out", [c.T, c.DM], DT.bfloat16,
                         kind="ExternalOutput").ap()

    with tile.TileContext(nc) as tc, ExitStack() as ctx:
        _body(ctx, tc, c, xT, wq, wk, wv, biases, wo, identd, maskb, out)
    nc.compile()
    names = ["xT", "wq", "wk", "wv", "biases", "wo", "ident"]
    if c.mode == "bias":
        names.append("maskb")
    return nc, names


def _body(ctx, tc, c: Cfg, xT, wq, wk, wv, biases, wo, identd, maskb, out):
    nc = tc.nc
    f32 = DT.float32
    bf16 = DT.bfloat16
    mmdt = c.mmdt
    causal = c.mode == "causal"
    scale = 1.0 / math.sqrt(c.HD)
    NHP = c.NH // 2                      # head pairs per core
    HD1 = c.HD + 1
    W2 = min(1024, c.T)                  # qk projection block width
    NB = c.T // W2                       # qk projection blocks
    VG = min(c.TC, max(1, 1024 // c.NHD))  # t-chunks per v psum tile
    NVT = c.TC // VG

    const = ctx.enter_context(tc.tile_pool(name="const", bufs=1))
    big = ctx.enter_context(tc.tile_pool(name="big", bufs=1))
    # lag-1 pipeline: E tiles of two consecutive units are alive at once
    # (worst case 2 * kmax_max/2 * 2 = 32 for causal T=2048)
    epool = ctx.enter_context(
        tc.tile_pool(name="E", bufs=(8 * c.QT if causal else c.TC + 4)))
    rpool = ctx.enter_context(tc.tile_pool(name="r", bufs=8))
    # PSUM: 3 x [128,1024] (6 banks) + 2 x [128,260] (2 banks) = 8 banks
    ps_mm = ctx.enter_context(tc.tile_pool(name="psmm", bufs=3, space="PSUM"))
    ps_pv = ctx.enter_context(tc.tile_pool(name="pspv", bufs=2, space="PSUM"))
    ostage = ctx.enter_context(tc.tile_pool(name="ostage", bufs=3))
    bias_pool = None
    if c.mode == "bias":
        bias_pool = ctx.enter_context(tc.tile_pool(name="maskb", bufs=4))

    # ---- input DMAs: qk weights first, then xT in pieces (the first score
    # block only needs t<512), with everything ordered by first use so the
    # sync queue's ~0.8us/descriptor serialization doesn't delay the PE ----
    w_sbs = {}
    for nm, w in (("wq", wq), ("wk", wk), ("wv", wv)):
        w_sbs[nm] = big.tile([128, c.KC, c.NHD], mmdt, tag=nm, name=nm)
    wo_sb = big.tile([128, c.MC, c.DM], mmdt, tag="wo")
    xT_sb = big.tile([128, c.KC, c.T], mmdt, tag="xT")
    xTd = xT.rearrange("(c p) t -> p c t", p=128)
    bias_sb = const.tile([128, 2 * c.MC + c.NHD], f32, tag="bias")
    bq_sb = bias_sb[:, 0:c.MC]
    bk_sb = bias_sb[:, c.MC:2 * c.MC]
    bvb_sb = bias_sb[:, 2 * c.MC:2 * c.MC + c.NHD]

    wqd = wq.rearrange("(c p) n -> p c n", p=128)
    wkd = wk.rearrange("(c p) n -> p c n", p=128)
    # sync queue: exactly what the prelude + early score units need, in
    # first-use order (the first tile needs wq/wk m=0 halves + xT[:, :, 0:W2])
    nc.sync.dma_start(out=w_sbs["wq"][:, :, 0:128], in_=wqd[:, :, 0:128])
    nc.sync.dma_start(out=w_sbs["wk"][:, :, 0:128], in_=wkd[:, :, 0:128])
    xcuts = ([0, c.T // 4, c.T // 2, 3 * c.T // 4, c.T]
             if c.T >= 2048 else [0, c.T])
    for x0, x1 in zip(xcuts[:2], xcuts[1:3]):
        nc.sync.dma_start(out=xT_sb[:, :, x0:x1], in_=xTd[:, :, x0:x1])
    nc.sync.dma_start(out=w_sbs["wq"][:, :, 128:c.NHD],
                      in_=wqd[:, :, 128:c.NHD])
    nc.sync.dma_start(out=w_sbs["wk"][:, :, 128:c.NHD],
                      in_=wkd[:, :, 128:c.NHD])
    nc.sync.dma_start(out=w_sbs["wv"][:],
                      in_=wv.rearrange("(c p) n -> p c n", p=128))
    nc.sync.dma_start(out=bias_sb[:], in_=biases)
    for x0, x1 in zip(xcuts[2:], xcuts[3:]):
        nc.sync.dma_start(out=xT_sb[:, :, x0:x1], in_=xTd[:, :, x0:x1])
    nc.sync.dma_start(out=wo_sb[:],
                      in_=wo.rearrange("(c p) n -> p c n", p=128))
    wq_sb, wk_sb, wv_sb = w_sbs["wq"], w_sbs["wk"], w_sbs["wv"]

    # causal mask block for diagonal tiles: tri[k, j] = 0 if j >= k else NEG
    tri = const.tile([128, 128], f32, tag="tri")
    nc.gpsimd.memset(tri[:], 0.0)
    nc.gpsimd.affine_select(
        out=tri[:], in_=tri[:],
        compare_op=ALU.is_ge, fill=NEG,
        base=0, channel_multiplier=-1, pattern=[[1, 128]],
    )
    # identity (bf16) for PE transposes of the attention output (host input)
    ident = const.tile([128, 128], bf16, tag="ident")
    nc.sync.dma_start(out=ident[:], in_=identd)

    # k64: qT packed like kT ([128, MC, T], head pair per 128 partitions) and
    # scores contract K=64 in a PE quadrant. Otherwise qT is zero-padded per
    # head ([128, NH, T]) so score matmuls run full-K=128 against the natural
    # two-head kT chunk (the other head's rows hit zeros).
    if c.k64:
        qT_z = big.tile([128, c.MC, c.T], mmdt, tag="qT")
    else:
        qT_z = big.tile([128, c.NH, c.T], mmdt, tag="qT")
        nc.vector.memset(qT_z[:], 0.0)
    kT_sb = big.tile([128, c.MC, c.T], mmdt, tag="kT")
    v_sb = big.tile([128, c.TC, c.NH, HD1], bf16, tag="v")
    nc.vector.memset(v_sb[:, :, :, c.HD:HD1], 1.0)

    a_sb = big.tile([128, c.TC, c.NH, c.HD], bf16, tag="a")
    aT_sb = big.tile([128, c.MC, c.T], bf16, tag="aT")

    # ---- projection emitters (used as PE filler inside the attention) ----
    def emit_qk_tile(m, which, n):
        w_sb, b_sb = (wq_sb, bq_sb) if which == "q" else (wk_sb, bk_sb)
        ps = ps_mm.tile([128, 1024], f32, tag="mm")
        for d in range(W2 // 512):
            for k in range(c.KC):
                nc.tensor.matmul(
                    ps[:, d * 512:(d + 1) * 512],
                    lhsT=w_sb[:, k, m * 128:(m + 1) * 128],
                    rhs=xT_sb[:, k, n * W2 + d * 512:n * W2 + (d + 1) * 512],
                    start=(k == 0), stop=(k == c.KC - 1),
                )
        sl = slice(n * W2, (n + 1) * W2)
        if which == "k":
            nc.vector.tensor_scalar_add(
                kT_sb[:, m, sl], ps[:, 0:W2], b_sb[:, m:m + 1],
            )
        elif c.k64:
            nc.vector.tensor_scalar_add(
                qT_z[:, m, sl], ps[:, 0:W2], b_sb[:, m:m + 1],
            )
        else:
            nc.vector.tensor_scalar_add(
                qT_z[0:64, 2 * m, sl], ps[0:64, 0:W2], b_sb[0:64, m:m + 1],
            )
            nc.vector.tensor_scalar_add(
                qT_z[64:128, 2 * m + 1, sl], ps[64:128, 0:W2],
                b_sb[64:128, m:m + 1],
            )

    def emit_v_tile(tg):
        # v in normal layout, augmented with a ones column per head;
        # VG t-chunks share one psum tile.
        ps = ps_mm.tile([128, 1024], f32, tag="mm")
        for d in range(VG):
            t = tg * VG + d
            for k in range(c.KC):
                nc.tensor.matmul(
                    ps[:, d * c.NHD:(d + 1) * c.NHD],
                    lhsT=xT_sb[:, k, t * 128:(t + 1) * 128],
                    rhs=wv_sb[:, k, :],
                    start=(k == 0), stop=(k == c.KC - 1),
                )
        for d in range(VG):
            t = tg * VG + d
            nc.vector.tensor_tensor(
                out=v_sb[:, t, :, 0:c.HD],
                in0=ps[:, d * c.NHD:(d + 1) * c.NHD].rearrange(
                    "p (h d) -> p h d", d=c.HD),
                in1=bvb_sb.rearrange("p (h d) -> p h d", d=c.HD),
                op=ALU.add,
            )

    # ---- attention unit emitters ----
    def emit_scores(g, hp, fillers):
        """Score matmuls + exp for unit (g, hp); fillers (projection-tile
        thunks) are spread between score tiles to keep the PE fed while the
        ACT engine works through the exp backlog."""
        fillers = list(fillers)
        kmax = (g + 1) * c.QT if causal else c.TC
        assert kmax % 2 == 0
        tiles = [(kp, hl) for kp in range(kmax // 2) for hl in range(2)]
        etiles = {}
        nfill = len(fillers)
        done_f = 0
        for idx, (kp, hl) in enumerate(tiles):
            h = 2 * hp + hl
            ps = ps_mm.tile([128, 1024], f32, tag="mm")
            lo0 = 0
            for d in range(2):
                kc = 2 * kp + d
                off = (kc - g * c.QT) * 128 if causal else -1
                lo = max(0, off)
                if d == 0:
                    lo0 = lo
                if c.k64:
                    hs = 64 * hl
                    nc.tensor.matmul(
                        ps[:, d * 512 + lo:d * 512 + c.QW],
                        lhsT=kT_sb[hs:hs + 64, hp, kc * 128:(kc + 1) * 128],
                        rhs=qT_z[hs:hs + 64, hp,
                                 g * c.QW + lo:(g + 1) * c.QW],
                        start=True, stop=True,
                    )
                else:
                    nc.tensor.matmul(
                        ps[:, d * 512 + lo:d * 512 + c.QW],
                        lhsT=kT_sb[:, hp, kc * 128:(kc + 1) * 128],
                        rhs=qT_z[:, h, g * c.QW + lo:(g + 1) * c.QW],
                        start=True, stop=True,
                    )
                if causal:
                    if off >= 0:
                        nc.vector.tensor_tensor(
                            out=ps[:, d * 512 + off:d * 512 + off + 128],
                            in0=ps[:, d * 512 + off:d * 512 + off + 128],
                            in1=tri[:], op=ALU.add,
                        )
                elif c.mode == "bias":
                    mb = bias_pool.tile([128, c.QW], f32, tag="mb")
                    nc.sync.dma_start(
                        out=mb[:],
                        in_=maskb[kc * 128:(kc + 1) * 128,
                                  g * c.QW:(g + 1) * c.QW],
                    )
                    nc.vector.tensor_tensor(
                        out=ps[:, d * 512:d * 512 + c.QW],
                        in0=ps[:, d * 512:d * 512 + c.QW],
                        in1=mb[:], op=ALU.add,
                    )
            et = epool.tile([128, 1024], bf16, tag="E")
            # exp only the columns PV will read (cols < lo0 are fully masked
            # for both chunks; stale et/psum bytes there are never consumed)
            nc.scalar.activation(et[:, lo0:1024], ps[:, lo0:1024],
                                 AF.Exp, scale=scale)
            etiles[(hl, kp)] = et
            want = ((idx + 1) * nfill) // len(tiles)
            while done_f < want:
                fillers[done_f]()
                done_f += 1
        while done_f < nfill:
            fillers[done_f]()
            done_f += 1
        return etiles

    def emit_pv(g, hp, etiles, qt_cb=None):
        """PV matmuls + softmax normalization for unit (g, hp). QT per-q-tile
        pv accumulators share one psum tile so the reciprocals batch. For the
        last head pair, transpose a -> aT on the PE; qt_cb(j) (flush path)
        emits the out-projection for each q tile right after its transpose."""
        last = hp == NHP - 1
        for hl in range(2):
            h = 2 * hp + hl
            # QT pv accumulators padded to 128 f32 each share one psum bank
            psv = ps_pv.tile([128, c.QT, 128], f32, tag="pv")
            for j in range(c.QT):
                qt = g * c.QT + j
                kn = qt + 1 if causal else c.TC
                for kc in range(kn):
                    kp, d = divmod(kc, 2)
                    nc.tensor.matmul(
                        psv[:, j, 0:HD1],
                        lhsT=etiles[(hl, kp)][
                            :, d * 512 + j * 128:d * 512 + (j + 1) * 128],
                        rhs=v_sb[:, kc, h, :],
                        start=(kc == 0), stop=(kc == kn - 1),
                    )
            r = rpool.tile([128, c.QT], f32, tag="r")
            nc.vector.reciprocal(r[:], psv[:, :, c.HD:HD1])
            for j in range(c.QT):
                qt = g * c.QT + j
                nc.vector.tensor_scalar_mul(
                    a_sb[:, qt, h, :], psv[:, j, 0:c.HD],
                    r[:, j:j + 1],
                )
        if last:
            # PE-transpose a -> aT (head pair ci per 128-wide tile); avoids
            # the DRAM round-trip DMA transpose and its end-of-group stall
            for j in range(c.QT):
                qt = g * c.QT + j
                for ci in range(c.MC):
                    # reuse the pv psum buffers (bf16 view) for the transpose
                    pts = ps_pv.tile([128, c.QT, 128], f32, tag="pv",
                                     name="pt")
                    pt = pts.bitcast(bf16)[:, 0, 0:128]
                    nc.tensor.transpose(
                        pt, a_sb[:, qt, 2 * ci:2 * ci + 2, :], ident[:],
                    )
                    nc.vector.tensor_copy(
                        aT_sb[:, ci, qt * 128:(qt + 1) * 128], pt,
                    )
                if qt_cb is not None:
                    qt_cb(j)

    def emit_outproj_tile(g, j, flush=False):
        t = g * c.QT + j
        ps = ps_mm.tile([128, 1024], f32, tag="mm")
        for d in range(c.DM // 512):
            for ci in range(c.MC):
                nc.tensor.matmul(
                    ps[:, d * 512:(d + 1) * 512],
                    lhsT=aT_sb[:, ci, t * 128:(t + 1) * 128],
                    rhs=wo_sb[:, ci, d * 512:(d + 1) * 512],
                    start=(ci == 0), stop=(ci == c.MC - 1),
                )
        ot = ostage.tile([128, c.DM], bf16, tag="o")
        # at the flush ACT is idle: alternate DVE/ACT so casts overlap
        if flush and j % 2 == 1:
            nc.scalar.copy(ot[:], ps[:, 0:c.DM])
        else:
            nc.vector.tensor_copy(ot[:], ps[:, 0:c.DM])
        nc.sync.dma_start(
            out=out[t * 128:(t + 1) * 128, :], in_=ot[:],
        )

    def outproj_thunks(g):
        return [lambda g=g, j=j: emit_outproj_tile(g, j)
                for j in range(c.QT)]

    # ---- schedule ----
    def qk_thunks(m, n):
        return [lambda m=m, n=n: emit_qk_tile(m, "q", n),
                lambda m=m, n=n: emit_qk_tile(m, "k", n)]

    def v_thunk(tg):
        return [lambda tg=tg: emit_v_tile(tg)]

    if causal and c.QG == 4 and NVT == 4 and NHP == 2 and NB == 2:
        # group order [1,3,2,0]: g=1 starts fast (needs only the first qk
        # block), g=0 (cheapest) last to minimize the serial tail.
        g_seq = [1, 3, 2, 0]
        prelude = [(0, 0)]
        # fillers ordered by input-DMA arrival: v tiles unblock before the
        # second-half qk blocks (which need the tail of xT)
        fill = {
            0: qk_thunks(1, 0) + v_thunk(0),
            1: v_thunk(1) + qk_thunks(0, 1),
            2: v_thunk(2) + qk_thunks(1, 1),
            3: v_thunk(3),
        }
        lag = 1
    else:
        g_seq = list(range(c.QG))
        prelude = [(m, n) for m in range(NHP) for n in range(NB)]
        fill = {0: [t for tg in range(NVT) for t in v_thunk(tg)]}
        lag = 0

    for m, n in prelude:
        emit_qk_tile(m, "q", n)
        emit_qk_tile(m, "k", n)

    units = [(g, hp) for g in g_seq for hp in range(NHP)]
    pending_pv = []     # (g, hp, etiles) awaiting PV emission
    extra_fill = []     # outproj tile thunks, interleaved into next scores
    for i, (g, hp) in enumerate(units):
        etiles = emit_scores(g, hp, fill.get(i, []) + extra_fill)
        extra_fill = []
        pending_pv.append((g, hp, etiles))
        if len(pending_pv) > lag:
            pg, php, pet = pending_pv.pop(0)
            emit_pv(pg, php, pet)
            if php == NHP - 1:
                extra_fill = outproj_thunks(pg)
    for pg, php, pet in pending_pv:
        for t in extra_fill:
            t()
        extra_fill = []
        if php == NHP - 1:
            # flush: out-projection of each q tile rides right behind its
            # aT transpose so the final casts/DMAs start as early as possible
            emit_pv(pg, php, pet,
                    qt_cb=lambda j, pg=pg: emit_outproj_tile(pg, j, flush=True))
        else:
            emit_pv(pg, php, pet)
    for t in extra_fill:
        t()


# ---------------------------------------------------------------------------
# host side
# ---------------------------------------------------------------------------

_CACHE: dict = {}


def _get_program(cfg: Cfg):
    key = cfg
    if key not in _CACHE:
        _CACHE[key] = build_program(cfg)
    return _CACHE[key]


def _mask_mode(mask: np.ndarray, T: int) -> str:
    m = (np.asarray(mask).reshape(T, T) != 0)
    if m.all():
        return "full"
    if np.array_equal(m, np.tril(np.ones((T, T), dtype=bool))):
        return "causal"
    return "bias"


def make_in_maps(cfg: Cfg, x, W_qkv, b_qkv, W_out, mask=None):
    """Slice full inputs into the 8 per-core input dicts."""
    c = cfg
    npmm = c.npmm
    B = x.shape[0]
    n_hg = N_CORES // B                      # head groups per batch
    in_maps = []
    maskb = None
    if c.mode == "bias":
        m = (np.asarray(mask).reshape(c.T, c.T) != 0)
        maskb = np.where(m, np.float32(0), np.float32(NEG)).T.copy()
    for core in range(N_CORES):
        b, hg = divmod(core, n_hg)
        col0 = hg * c.NHD
        xT = np.ascontiguousarray(x[b].T).astype(npmm)
        wq_ = np.ascontiguousarray(W_qkv[:, 0 * c.DM + col0:0 * c.DM + col0 + c.NHD]).astype(npmm)
        wk_ = np.ascontiguousarray(W_qkv[:, 1 * c.DM + col0:1 * c.DM + col0 + c.NHD]).astype(npmm)
        wv_ = np.ascontiguousarray(W_qkv[:, 2 * c.DM + col0:2 * c.DM + col0 + c.NHD]).astype(npmm)
        bq_ = np.ascontiguousarray(
            b_qkv[0 * c.DM + col0:0 * c.DM + col0 + c.NHD].reshape(c.MC, 128).T
        ).astype(np.float32)
        bk_ = np.ascontiguousarray(
            b_qkv[1 * c.DM + col0:1 * c.DM + col0 + c.NHD].reshape(c.MC, 128).T
        ).astype(np.float32)
        bv_ = b_qkv[2 * c.DM + col0:2 * c.DM + col0 + c.NHD].astype(np.float32)
        bvb_ = np.ascontiguousarray(np.broadcast_to(bv_, (128, c.NHD)))
        biases_ = np.ascontiguousarray(
            np.concatenate([bq_, bk_, bvb_], axis=1))
        wo_ = np.ascontiguousarray(W_out[col0:col0 + c.NHD, :]).astype(npmm)
        im = dict(xT=xT, wq=wq_, wk=wk_, wv=wv_, biases=biases_,
                  wo=wo_, ident=np.eye(128, dtype=ml_dtypes.bfloat16))
        if c.mode == "bias":
            im["maskb"] = maskb
        in_maps.append(im)
    return in_maps


def run_sharded(cfg: Cfg, x, W_qkv, b_qkv, W_out, b_out, mask=None, **kw):
    """Run the SPMD program on 8 cores and assemble the full output."""
    nc, _names = _get_program(cfg)
    in_maps = make_in_maps(cfg, x, W_qkv, b_qkv, W_out, mask)
    res = bass_utils.run_bass_kernel_spmd(
        nc, in_maps, core_ids=list(range(N_CORES)), **kw,
    )
    outs = [np.asarray(r["out"]).astype(np.float32) for r in res.results]
    B = x.shape[0]
    n_hg = N_CORES // B
    y = np.stack([
        np.sum(outs[b * n_hg:(b + 1) * n_hg], axis=0) for b in range(B)
    ]) + b_out.astype(np.float32)
    return y.astype(np.float32), res


def kernel(x, W_qkv, b_qkv, W_out, b_out, mask):
    x = np.asarray(x, dtype=np.float32)
    W_qkv = np.asarray(W_qkv, dtype=np.float32)
    b_qkv = np.asarray(b_qkv, dtype=np.float32)
    W_out = np.asarray(W_out, dtype=np.float32)
    b_out = np.asarray(b_out, dtype=np.float32)
    B, T, DM = x.shape
    mode = _mask_mode(mask, T)
    cfg = Cfg(T=T, DM=DM, mode=mode, mm=os.environ.get("MHA_MM_DT", "bf16"))
    y, _ = run_sharded(cfg, x, W_qkv, b_qkv, W_out, b_out, mask)
    return y
